# revision 8
# baseline (speedup 1.0000x reference)
"""Multi-head attention (B=4, S=2048, D=768, H=12) on 8 trn2 NeuronCores.

Sharding: core c handles batch b = c//2 and head-half hh = c%2 (6 heads,
384 features). Each core computes a partial output [2048, 768] (its 6 heads'
contribution through the output projection, un-biased); the host sums the
two partials per batch and adds OW_b plus the V-bias constant
(softmax rows sum to 1, so the V bias contributes wo_slice @ vb per token).

On-chip dataflow (all matmuls bf16 with fp32 PSUM accumulation):
  x [2048,768] --PE transpose--> xT [768,2048]
  W[qkv] rows --PE transpose--> wT [768,384]; QT/KT/VT = wT.T @ xT  [384,2048]
  (Q/K biases added per-partition during PSUM evac)
  VT --PE transpose--> v1 [2048, 6*(64+1)] with a ones column per head
  per (head, kchunk): S^T chunk [128k, 2048q] = KT_slice.T @ QT_slice
    exp fused into ACT evac (scale=1/8); P^T bf16
    attn^T [65, 1024] += v1_slice.T @ P^T   (row 64 = softmax denominator)
  normalize: recip(denom) -> DMA-free partition broadcast -> DVE mul -> attnT
  out [128tok, 768] = attnT_chunk.T @ woT  (wo transposed on chip)
"""

import numpy as np

import concourse.bass as bass
import concourse.mybir as mybir
import concourse.tile as tile
from concourse import masks
from concourse.bass_utils import run_bass_kernel_spmd

F32 = mybir.dt.float32
BF16 = mybir.dt.bfloat16
AF = mybir.ActivationFunctionType

B, S, D = 4, 2048, 768
H, HD = 12, 64
N_CORES = 8
HEADS_PER_CORE = 6          # 12 heads / 2 cores per batch
FS = HEADS_PER_CORE * HD    # 384 features per core
SCALE = 1.0 / np.sqrt(HD)   # 0.125

KT16 = S // 128             # 16 token tiles of 128
QB = 1024                   # q-block (free dim of S^T chunks)
NQB = S // QB               # 2


def split_waits(nc, cap=1):
    """walrus rejects instructions carrying >2 sync waits; the TileContext
    final drain is emitted post-lowering and can carry many. Hoist excess
    waits onto preceding same-engine NOPs (1 wait each)."""
    f = nc.m.functions[0]
    for bb in f.blocks:
        insts = list(bb.instructions)
        new = []
        changed = False
        for inst in insts:
            si = inst.sync_info
            if si is not None and si.on_wait is not None and len(si.on_wait) > cap:
                waits = list(si.on_wait)
                keep = waits[-cap:]
                extra = waits[:-cap]
                for j, w in enumerate(extra):
                    nop = mybir.InstNoOp(
                        name=f"{inst.name}-wsplit{j}",
                        engine=inst.engine,
                        ins=[], outs=[],
                        sync_info=mybir.SyncInfo(on_wait=[w], on_update=[]),
                    )
                    new.append(nop)
                    changed = True
                inst.sync_info = mybir.SyncInfo(
                    on_wait=keep, on_update=list(si.on_update or [])
                )
            new.append(inst)
        if changed:
            bb.instructions = new


def build_nc():
    nc = bass.Bass()

    x_ext = nc.dram_tensor("xb", [S, D], F32, kind="ExternalInput")
    wq_ext = nc.dram_tensor("wq", [FS, D], F32, kind="ExternalInput")
    wk_ext = nc.dram_tensor("wk", [FS, D], F32, kind="ExternalInput")
    wv_ext = nc.dram_tensor("wv", [FS, D], F32, kind="ExternalInput")
    wo_ext = nc.dram_tensor("wo", [D, FS], F32, kind="ExternalInput")
    bq_ext = nc.dram_tensor("bq", [FS], F32, kind="ExternalInput")
    bk_ext = nc.dram_tensor("bk", [FS], F32, kind="ExternalInput")
    out_ext = nc.dram_tensor("out_part", [S, D], F32, kind="ExternalOutput")

    with tile.TileContext(nc) as tc:
        with tc.tile_pool(name="persist", bufs=1) as P:
            # persistent SBUF arrays
            ident = P.tile([128, 128], F32, tag="ident", name="ident")
            masks.make_identity(nc, ident[:])
            ident_b = P.tile([128, 128], BF16, tag="ident_b", name="ident_b")
            masks.make_identity(nc, ident_b[:])

            xT = [P.tile([128, S], BF16, tag=f"xT{j}", name=f"xT{j}") for j in range(6)]
            wT = {p: [P.tile([128, FS], BF16, tag=f"wT{p}{j}", name=f"wT{p}{j}") for j in range(6)]
                  for p in "qkv"}
            woT = [P.tile([128, D], BF16, tag=f"woT{j}", name=f"woT{j}") for j in range(3)]
            QT = [P.tile([128, S], BF16, tag=f"QT{m}", name=f"QT{m}") for m in range(3)]
            KT = [P.tile([128, S], BF16, tag=f"KT{m}", name=f"KT{m}") for m in range(3)]
            VT = [P.tile([128, S], BF16, tag=f"VT{m}", name=f"VT{m}") for m in range(3)]
            # per head a 128-col block: V in one 64-col half, ones in the
            # other (ones half opposite to the head's attnT partition half,
            # so the softmax denominator lands on the free partition half)
            v1 = [P.tile([128, HEADS_PER_CORE * 128], BF16, tag=f"v1{t}", name=f"v1{t}")
                  for t in range(KT16)]
            attnT = [P.tile([128, S], BF16, tag=f"attnT{m}", name=f"attnT{m}") for m in range(3)]
            qb_sb = P.tile([128, 3], F32, tag="qb", name="qb_sb")
            kb_sb = P.tile([128, 3], F32, tag="kb", name="kb_sb")

            nc.sync.dma_start(qb_sb[:], bq_ext.rearrange("(j p) -> p j", p=128))
            nc.sync.dma_start(kb_sb[:], bk_ext.rearrange("(j p) -> p j", p=128))

            # ---------------- Phase A/B: transposes + projections ----------
            with (
                tc.tile_pool(name="ld", bufs=3) as L,
                tc.tile_pool(name="tp", bufs=4, space="PSUM") as TP,
                tc.tile_pool(name="pj", bufs=3, space="PSUM") as PJ,
            ):
                # x -> xT (bf16)
                for t in range(KT16):
                    xt = L.tile([128, D], F32, tag="xload", name="xload")
                    nc.sync.dma_start(xt[:], x_ext[t * 128:(t + 1) * 128, :])
                    for j in range(6):
                        ps = TP.tile([128, 128], F32, tag="tp", name="tp")
                        nc.tensor.transpose(
                            ps[:], xt[:, j * 128:(j + 1) * 128], ident[:])
                        nc.vector.tensor_copy(
                            xT[j][:, t * 128:(t + 1) * 128], ps[:])

                # weights -> wT (bf16)
                for p, ext in (("q", wq_ext), ("k", wk_ext), ("v", wv_ext)):
                    for r in range(3):
                        wt = L.tile([128, D], F32, tag="wload", name="wload")
                        nc.sync.dma_start(wt[:], ext[r * 128:(r + 1) * 128, :])
                        for j in range(6):
                            ps = TP.tile([128, 128], F32, tag="tp", name="tp")
                            nc.tensor.transpose(
                                ps[:], wt[:, j * 128:(j + 1) * 128], ident[:])
                            nc.vector.tensor_copy(
                                wT[p][j][:, r * 128:(r + 1) * 128], ps[:])

                # wo [768, 384] -> woT [384, 768] (bf16)
                for r in range(6):
                    wt = L.tile([128, FS], F32, tag="woload", name="woload")
                    nc.sync.dma_start(wt[:], wo_ext[r * 128:(r + 1) * 128, :])
                    for j in range(3):
                        ps = TP.tile([128, 128], F32, tag="tp", name="tp")
                        nc.tensor.transpose(
                            ps[:], wt[:, j * 128:(j + 1) * 128], ident[:])
                        nc.vector.tensor_copy(
                            woT[j][:, r * 128:(r + 1) * 128], ps[:])

                # QKV projections: [feat, tok] = wT.T @ xT
                for p, dsts, bias in (("q", QT, qb_sb), ("k", KT, kb_sb),
                                      ("v", VT, None)):
                    for m in range(3):
                        for s4 in range(4):
                            ps = PJ.tile([128, 512], F32, tag="pj", name="pj")
                            for kc in range(6):
                                nc.tensor.matmul(
                                    ps[:],
                                    wT[p][kc][:, m * 128:(m + 1) * 128],
                                    xT[kc][:, s4 * 512:(s4 + 1) * 512],
                                    start=(kc == 0), stop=(kc == 5),
                                )
                            dst = dsts[m][:, s4 * 512:(s4 + 1) * 512]
                            if bias is not None:
                                nc.vector.tensor_scalar_add(
                                    dst, ps[:], bias[:, m:m + 1])
                            else:
                                nc.vector.tensor_copy(dst, ps[:])

                # VT -> v1 (per-head: V block in the head's partition half,
                # ones in the other half)
                for t in range(KT16):
                    nc.vector.memset(v1[t][:], 1.0)
                for h in range(HEADS_PER_CORE):
                    mt, po = h // 2, (h % 2) * 64
                    for t in range(KT16):
                        ps = TP.tile([128, 64], BF16, tag="tp", name="tpv")
                        nc.tensor.transpose(
                            ps[:],
                            VT[mt][po:po + 64, t * 128:(t + 1) * 128],
                            ident_b[po:po + 64, po:po + 64],
                        )
                        nc.vector.tensor_copy(
                            v1[t][:, h * 128 + po:h * 128 + po + 64], ps[:])

            # ---------------- Phase C: attention ---------------------------
            with (
                tc.tile_pool(name="sp", bufs=2, space="PSUM") as SP,
                tc.tile_pool(name="ap0", bufs=1, space="PSUM") as AP0,
                tc.tile_pool(name="ap1", bufs=1, space="PSUM") as AP1,
                tc.tile_pool(name="pw", bufs=3) as PW,
                tc.tile_pool(name="nw", bufs=2) as NW,
            ):
                for h in range(HEADS_PER_CORE):
                    mt, po = h // 2, (h % 2) * 64
                    pd = 64 - po  # denominator partition half (opposite po)
                    at = [AP0.tile([128, QB], F32, tag="at0", name="at0"),
                          AP1.tile([128, QB], F32, tag="at1", name="at1")]
                    for kc in range(KT16):
                        for qb in range(NQB):
                            sps = SP.tile([128, QB], F32, tag="s", name="s")
                            for j in range(QB // 512):
                                nc.tensor.matmul(
                                    sps[:, j * 512:(j + 1) * 512],
                                    KT[mt][po:po + 64, kc * 128:(kc + 1) * 128],
                                    QT[mt][po:po + 64,
                                           qb * QB + j * 512:qb * QB + (j + 1) * 512],
                                    start=True, stop=True,
                                )
                            pt = PW.tile([128, QB], BF16, tag="p", name="p")
                            nc.scalar.activation(pt[:], sps[:], AF.Exp, scale=SCALE)
                            for j in range(QB // 512):
                                nc.tensor.matmul(
                                    at[qb][:, j * 512:(j + 1) * 512],
                                    v1[kc][:, h * 128:(h + 1) * 128],
                                    pt[:, j * 512:(j + 1) * 512],
                                    start=(kc == 0), stop=(kc == KT16 - 1),
                                )
                    for qb in range(NQB):
                        # attn rows live at partitions [po, po+64); the
                        # denominator (replicated via the 64 ones columns)
                        # at [pd, pd+64). DVE lanes are partition-locked, so
                        # recip on the denom half, DMA it across, multiply.
                        rc = NW.tile([128, QB], F32, tag="rc", name="rc")
                        nc.vector.reciprocal(
                            rc[pd:pd + 64, :], at[qb][pd:pd + 64, :])
                        nc.sync.dma_start(rc[po:po + 64, :], rc[pd:pd + 64, :])
                        nc.vector.tensor_mul(
                            attnT[mt][po:po + 64, qb * QB:(qb + 1) * QB],
                            at[qb][po:po + 64, :], rc[po:po + 64, :])

            # ---------------- Phase D: output projection --------------------
            with (
                tc.tile_pool(name="op", bufs=3, space="PSUM") as OP,
                tc.tile_pool(name="ow", bufs=3) as OW,
            ):
                for t in range(KT16):
                    ps = OP.tile([128, D], F32, tag="o", name="o")
                    for kc in range(3):
                        for (lo, hi) in ((0, 512), (512, 768)):
                            nc.tensor.matmul(
                                ps[:, lo:hi],
                                attnT[kc][:, t * 128:(t + 1) * 128],
                                woT[kc][:, lo:hi],
                                start=(kc == 0), stop=(kc == 2),
                            )
                    ot = OW.tile([128, D], F32, tag="ot", name="ot")
                    nc.vector.tensor_copy(ot[:], ps[:])
                    nc.sync.dma_start(out_ext[t * 128:(t + 1) * 128, :], ot[:])

    split_waits(nc)
    return nc


_NC_CACHE = None


def _get_nc():
    global _NC_CACHE
    if _NC_CACHE is None:
        _NC_CACHE = build_nc()
    return _NC_CACHE


def make_in_maps(x, QW_w, QW_b, KW_w, KW_b, VW_w, VW_b, OW_w, OW_b):
    f32 = lambda a: np.ascontiguousarray(np.asarray(a), dtype=np.float32)
    in_maps = []
    for c in range(N_CORES):
        b, hh = c // 2, c % 2
        sl = slice(hh * FS, (hh + 1) * FS)
        in_maps.append({
            "xb": f32(x[b]),
            "wq": f32(QW_w[sl, :]),
            "wk": f32(KW_w[sl, :]),
            "wv": f32(VW_w[sl, :]),
            "wo": f32(OW_w[:, sl]),
            "bq": f32(QW_b[sl]),
            "bk": f32(KW_b[sl]),
        })
    return in_maps


def kernel(x, QW_w, QW_b, KW_w, KW_b, VW_w, VW_b, OW_w, OW_b):
    nc = _get_nc()
    in_maps = make_in_maps(x, QW_w, QW_b, KW_w, KW_b, VW_w, VW_b, OW_w, OW_b)
    res = run_bass_kernel_spmd(nc, in_maps, list(range(N_CORES)))

    out = np.zeros((B, S, D), dtype=np.float32)
    OW_w = np.asarray(OW_w, dtype=np.float32)
    OW_b = np.asarray(OW_b, dtype=np.float32)
    VW_b = np.asarray(VW_b, dtype=np.float32)
    for c in range(N_CORES):
        b = c // 2
        out[b] += res.results[c]["out_part"]
    for b in range(B):
        # OW bias + V-bias routed through the output projection
        out[b] += OW_b + OW_w @ VW_b
    return out


# revision 17
# speedup vs baseline: 34.3416x; 34.3416x over previous
"""Multi-head attention (B=4, S=2048, D=768, H=12) on 8 trn2 NeuronCores.

Sharding: core c handles batch b = c//2 and head-half hh = c%2 (6 heads,
384 features). Each core computes a partial output [2048, 768] (its 6 heads'
contribution through the output projection, un-biased); the host sums the
two partials per batch and adds OW_b plus the V-bias constant
(softmax rows sum to 1, so the V bias contributes OW_w @ VW_b per token).

On-chip dataflow (all matmuls bf16 with fp32 PSUM accumulation):
  x/w loaded fp32 (batched DMAs), cast to bf16 on GpSimd, transposed via
  batched DMA-xbar transposes (3-D output APs -> one DMA per source tile)
  QT/KT/VT [feat, tok] = wT.T @ xT; Q/K biases added per-partition on evac
  per head a 128-col v1 block per k-chunk: V in the head's 64-partition
  half, ones in the other half (softmax denominator lands there)
  per (head, qblock, kchunk): S^T [128k, 1024q] = KT_slice.T @ QT_slice
    exp fused into the ACT PSUM evac (scale=1/8) -> P^T bf16
    attn psum [128, 1024] += v1_chunk.T @ P^T  (64 attn rows + 64 denom rows)
  normalize: evac psum, recip denom half, DMA across partition halves, mul
  out [128tok, 768] = attnT_chunk.T @ woT

Projections of feature-chunk m+1 overlap the (ScalarE-bound) attention of
heads 2m, 2m+1; PSUM budget: pj 2 + scores 4 + attn 2 = 8 banks.
DMA routing: SP HWDGE for loads/transposes, GpSimd SWDGE for stores and
the small normalize moves (the SEQ cost per dma_start is ~650ns, so DMA
count is kept low and split across the two sequencers).
"""

import numpy as np

import concourse.bass as bass
import concourse.mybir as mybir
import concourse.tile as tile
from concourse.bass_utils import run_bass_kernel_spmd

F32 = mybir.dt.float32
BF16 = mybir.dt.bfloat16
AF = mybir.ActivationFunctionType

B, S, D = 4, 2048, 768
H, HD = 12, 64
N_CORES = 8
HEADS_PER_CORE = 6          # 12 heads / 2 cores per batch
FS = HEADS_PER_CORE * HD    # 384 features per core
SCALE = 1.0 / np.sqrt(HD)   # 0.125

KT16 = S // 128             # 16 token tiles of 128
QB = 1024                   # q-block (free dim of S^T chunks)
NQB = S // QB               # 2


def split_waits(nc, cap=1):
    """walrus rejects instructions carrying >2 sync waits; the TileContext
    final drain is emitted post-lowering and can carry many. Hoist excess
    waits onto preceding same-engine NOPs (1 wait each)."""
    f = nc.m.functions[0]
    for bb in f.blocks:
        insts = list(bb.instructions)
        new = []
        changed = False
        for inst in insts:
            si = inst.sync_info
            if si is not None and si.on_wait is not None and len(si.on_wait) > cap:
                waits = list(si.on_wait)
                keep = waits[-cap:]
                extra = waits[:-cap]
                for j, w in enumerate(extra):
                    nop = mybir.InstNoOp(
                        name=f"{inst.name}-wsplit{j}",
                        engine=inst.engine,
                        ins=[], outs=[],
                        sync_info=mybir.SyncInfo(on_wait=[w], on_update=[]),
                    )
                    new.append(nop)
                    changed = True
                inst.sync_info = mybir.SyncInfo(
                    on_wait=keep, on_update=list(si.on_update or [])
                )
            new.append(inst)
        if changed:
            bb.instructions = new


def build_nc(reps=1, parts="prep,attn,out"):
    nc = bass.Bass()

    x_ext = nc.dram_tensor("xt", [D, S], F32, kind="ExternalInput")
    wq_ext = nc.dram_tensor("wqt", [D, FS], F32, kind="ExternalInput")
    wk_ext = nc.dram_tensor("wkt", [D, FS], F32, kind="ExternalInput")
    wv_ext = nc.dram_tensor("wvt", [D, FS], F32, kind="ExternalInput")
    wo_ext = nc.dram_tensor("wot", [FS, D], F32, kind="ExternalInput")
    bq_ext = nc.dram_tensor("bq", [FS], F32, kind="ExternalInput")
    bk_ext = nc.dram_tensor("bk", [FS], F32, kind="ExternalInput")
    out_ext = nc.dram_tensor("out_part", [S, D], F32, kind="ExternalOutput")

    with tile.TileContext(nc) as tc:
      for _rep in range(reps):
        with tc.tile_pool(name="persist", bufs=1) as P:
            # xT per d-chunk (tokens contiguous)
            xTc = [P.tile([128, S], BF16, name=f"xTc{j}") for j in range(6)]
            # wT block j (d-chunk) at cols j*FS
            wT = {p: P.tile([128, 6 * FS], BF16, name=f"wT{p}") for p in "qkv"}
            # woT block j (hd-chunk) at cols j*D
            woT = P.tile([128, 3 * D], BF16, name="woT")
            QT = [P.tile([128, S], BF16, name=f"QT{m}") for m in range(3)]
            KT = [P.tile([128, S], BF16, name=f"KT{m}") for m in range(3)]
            VT = [P.tile([128, S], BF16, name=f"VT{m}") for m in range(3)]
            # v1 per head: block for kchunk t at cols t*128; within a block
            # V fills cols po..po+64 (the head's attn-psum partition half),
            # the rest stays 1.0 (softmax denominator rows)
            v1 = [P.tile([128, KT16 * 128], BF16, name=f"v1h{h}")
                  for h in range(HEADS_PER_CORE)]
            attnT = {(m, q): P.tile([128, QB], BF16, name=f"attnT{m}_{q}")
                     for m in range(3) for q in range(NQB)}
            qb_sb = P.tile([128, 3], F32, name="qb_sb")
            kb_sb = P.tile([128, 3], F32, name="kb_sb")

            nc.sync.dma_start(qb_sb[:], bq_ext.rearrange("(j p) -> p j", p=128))
            nc.sync.dma_start(kb_sb[:], bk_ext.rearrange("(j p) -> p j", p=128))
            for h in range(HEADS_PER_CORE):
                (nc.vector if h % 2 else nc.gpsimd).memset(v1[h][:], 1.0)

            v13 = [t[:].rearrange("p (t q) -> p t q", t=KT16) for t in v1]

            with (
                tc.tile_pool(name="ld", bufs=2) as L,
                tc.tile_pool(name="cst", bufs=2) as C,
                tc.tile_pool(name="pj", bufs=2, space="PSUM") as PJ,
                tc.tile_pool(name="sp", bufs=2, space="PSUM") as SP,
                tc.tile_pool(name="atp", bufs=1, space="PSUM") as AT,
                tc.tile_pool(name="nw", bufs=2) as NW,
                tc.tile_pool(name="pw", bufs=3) as PW,
            ):
                # ---- phase A: inputs arrive pre-transposed; load + cast --
                for wi, (p, ext) in enumerate(
                        (("q", wq_ext), ("k", wk_ext), ("v", wv_ext))):
                    wt = L.tile([128, 6 * FS], F32, tag="wld", name="wload")
                    nc.scalar.dma_start(
                        wt[:].rearrange("p (j f) -> p j f", j=6),
                        ext[:].rearrange("(j p) f -> p j f", p=128))
                    ceng = nc.gpsimd if wi % 2 else nc.vector
                    ceng.tensor_copy(wT[p][:], wt[:])

                wt = L.tile([128, 3 * D], F32, tag="wold", name="woload")
                nc.scalar.dma_start(
                    wt[:].rearrange("p (j f) -> p j f", j=3),
                    wo_ext[:].rearrange("(j p) f -> p j f", p=128))
                nc.gpsimd.tensor_copy(woT[:], wt[:])

                for j in range(6):
                    xt = L.tile([128, S], F32, tag="xld", name="xload")
                    nc.sync.dma_start(xt[:], x_ext[j * 128:(j + 1) * 128, :])
                    ceng = nc.gpsimd if j % 2 else nc.vector
                    ceng.tensor_copy(xTc[j][:], xt[:])

                # ---- projections m interleaved with attention 2m, 2m+1 ---
                for m in range(3):
                    for p, dsts, bias in (("q", QT, qb_sb), ("k", KT, kb_sb),
                                          ("v", VT, None)):
                        for s4 in range(4):
                            ps = PJ.tile([128, 512], F32, tag="pj", name="pj")
                            for kc in range(6):
                                nc.tensor.matmul(
                                    ps[:],
                                    wT[p][:, kc * FS + m * 128:
                                          kc * FS + (m + 1) * 128],
                                    xTc[kc][:, s4 * 512:(s4 + 1) * 512],
                                    start=(kc == 0), stop=(kc == 5),
                                )
                            dst = dsts[m][:, s4 * 512:(s4 + 1) * 512]
                            if bias is not None:
                                nc.vector.tensor_scalar_add(
                                    dst, ps[:], bias[:, m:m + 1])
                            else:
                                nc.vector.tensor_copy(dst, ps[:])

                    for h in (2 * m, 2 * m + 1):
                        po = (h % 2) * 64
                        nc.sync.dma_start_transpose(
                            v13[h][:, :, po:po + 64],
                            VT[m][po:po + 64, :])

                    if "attn" not in parts:
                        continue
                    for h in (2 * m, 2 * m + 1):
                        po = (h % 2) * 64
                        pd = 64 - po
                        for qb in range(NQB):
                            at = AT.tile([128, QB], F32, tag="at", name="at")
                            for kc in range(KT16):
                                sps = SP.tile([128, QB], F32, tag="s", name="s")
                                for j in range(QB // 512):
                                    nc.tensor.matmul(
                                        sps[:, j * 512:(j + 1) * 512],
                                        KT[m][po:po + 64,
                                              kc * 128:(kc + 1) * 128],
                                        QT[m][po:po + 64,
                                              qb * QB + j * 512:
                                              qb * QB + (j + 1) * 512],
                                        start=True, stop=True,
                                    )
                                pt = PW.tile([128, QB], BF16, tag="p", name="p")
                                nc.scalar.activation(
                                    pt[:], sps[:], AF.Exp, scale=SCALE)
                                for j in range(QB // 512):
                                    nc.tensor.matmul(
                                        at[:, j * 512:(j + 1) * 512],
                                        v1[h][:, kc * 128:(kc + 1) * 128],
                                        pt[:, j * 512:(j + 1) * 512],
                                        start=(kc == 0), stop=(kc == KT16 - 1),
                                    )
                            # evac attn psum quickly (frees the psum slot),
                            # then normalize: recip on the denominator half,
                            # DMA it across partition halves, multiply.
                            ats = NW.tile([128, QB], F32, tag="ats", name="ats")
                            nc.vector.tensor_copy(ats[:], at[:])
                            rc = NW.tile([128, QB], F32, tag="rc", name="rc")
                            nc.vector.reciprocal(
                                rc[pd:pd + 64, :], ats[pd:pd + 64, :])
                            nc.gpsimd.dma_start(
                                rc[po:po + 64, :], rc[pd:pd + 64, :])
                            nc.vector.tensor_mul(
                                attnT[(m, qb)][po:po + 64, :],
                                ats[po:po + 64, :], rc[po:po + 64, :])

                # ---- output projection (PSUM from the pj pool, so it
                # overlaps the tail of attention) ----------------------
                if "out" not in parts:
                    continue
                for t in range(KT16):
                    ot = PW.tile([128, D], F32, tag="ot", name="ot")
                    for (lo, hi) in ((0, 512), (512, 768)):
                        ps = PJ.tile([128, hi - lo], F32, tag="pj", name="opj")
                        for kc in range(3):
                            nc.tensor.matmul(
                                ps[:],
                                attnT[(kc, t // 8)][:, (t % 8) * 128:
                                                    (t % 8 + 1) * 128],
                                woT[:, kc * D + lo:kc * D + hi],
                                start=(kc == 0), stop=(kc == 2),
                            )
                        nc.vector.tensor_copy(ot[:, lo:hi], ps[:])
                    nc.gpsimd.dma_start(out_ext[t * 128:(t + 1) * 128, :], ot[:])

    split_waits(nc)
    return nc


_NC_CACHE = None


def _get_nc():
    global _NC_CACHE
    if _NC_CACHE is None:
        _NC_CACHE = build_nc()
    return _NC_CACHE


def make_in_maps(x, QW_w, QW_b, KW_w, KW_b, VW_w, VW_b, OW_w, OW_b):
    f32 = lambda a: np.ascontiguousarray(np.asarray(a), dtype=np.float32)
    in_maps = []
    for c in range(N_CORES):
        b, hh = c // 2, c % 2
        sl = slice(hh * FS, (hh + 1) * FS)
        in_maps.append({
            "xt": f32(np.asarray(x[b]).T),
            "wqt": f32(np.asarray(QW_w)[sl, :].T),
            "wkt": f32(np.asarray(KW_w)[sl, :].T),
            "wvt": f32(np.asarray(VW_w)[sl, :].T),
            "wot": f32(np.asarray(OW_w)[:, sl].T),
            "bq": f32(QW_b[sl]),
            "bk": f32(KW_b[sl]),
        })
    return in_maps


def kernel(x, QW_w, QW_b, KW_w, KW_b, VW_w, VW_b, OW_w, OW_b):
    nc = _get_nc()
    in_maps = make_in_maps(x, QW_w, QW_b, KW_w, KW_b, VW_w, VW_b, OW_w, OW_b)
    res = run_bass_kernel_spmd(nc, in_maps, list(range(N_CORES)))

    out = np.zeros((B, S, D), dtype=np.float32)
    OW_w = np.asarray(OW_w, dtype=np.float32)
    OW_b = np.asarray(OW_b, dtype=np.float32)
    VW_b = np.asarray(VW_b, dtype=np.float32)
    for c in range(N_CORES):
        b = c // 2
        out[b] += res.results[c]["out_part"]
    for b in range(B):
        # OW bias + V-bias routed through the output projection
        out[b] += OW_b + OW_w @ VW_b
    return out
